# revision 1
# baseline (speedup 1.0000x reference)
"""CommNet forward kernel for 8 Trainium2 NeuronCores.

Reference computation (per sample of N=32 agents, batch B=16384):
    h   = relu(obs @ enc_w + enc_b)                    # [B,N,64]
    2x:  msg = (sum_n h - h)/31
         h   = relu(concat(h, msg) @ comm_w[r] + comm_b[r])
    hid = relu(h @ out_w1 + out_b1)
    q   = hid @ out_w2 + out_b2; q[avail==0] = -1e10

Device strategy (pure data parallel, batch split 8 ways):
  * activations feature-major [feat(part), row(free)]; four 512-row
    groups per 2048-row super-iteration, packed 2x2 into the PE array
    via tile_position (K=64, M=64 quadrants) so all 16 subarrays
    compute concurrently.  Groups at (p-half, f-half) positions
    (0,1)/(1,0) swap every matmul layer; 4 permuting layers = identity,
    so the out2 layout matches the obs layout.
  * comm round rewritten as h @ W_self + S @ W_sum with
    W_self = W_h - W_m/31, W_sum = W_m/31, S = per-sample agent sum.
    S comes from identity-weight matmuls with a step-0 (broadcast)
    output AP that accumulates the 32 agent columns of each sample into
    one PSUM column; the S @ W_sum term re-broadcasts S via a step-0
    rhs AP into the same accumulation group as the W_self matmul.
  * relu+bias fused into the PSUM->SBUF evacuation (DVE dual-op
    tensor_scalar for enc/out1, ScalarE activation for the rounds)
  * mask+final bias folded host-side into pen = where(avail, out_b2, -1e10);
    pen is added on the PE (identity-lhsT matmul accumulate) and the q
    bank evacuated with a ScalarE copy
  * host pre-packs obs into the feature-major layout and unpacks q
    (layout work is free on host; the device does all the FLOPs)
"""

import contextlib
import sys

import numpy as np

sys.path.insert(0, "/opt/trn_rl_repo")

import ml_dtypes  # noqa: E402

B, N, OBS, H, A, NR = 16384, 32, 64, 64, 16, 2
NCORES = 8
RPC = B * N // NCORES   # rows per core = 65536

SUP = 2048              # rows per super-iteration (4 groups of 512)
GRP = 512               # rows per group (one fp32 PSUM bank)
NSUP = RPC // SUP
NS_G = GRP // N         # samples per group = 16
NS_H = 2 * NS_G         # samples per partition-half per super = 32

_cache = {}


def _build_device_program():
    import concourse.bacc as bacc
    import concourse.mybir as mybir
    from concourse import tile

    F32 = mybir.dt.float32
    BF16 = mybir.dt.bfloat16

    nc = bacc.Bacc("TRN2", target_bir_lowering=False, debug=False)

    obs_d = nc.dram_tensor("obs_pk", [NSUP, 128, SUP // 2], BF16, kind="ExternalInput")
    pen_d = nc.dram_tensor("pen_pk", [NSUP // 2, 128, GRP], F32, kind="ExternalInput")
    q_d = nc.dram_tensor("q_pk", [NSUP // 2, 128, GRP], BF16, kind="ExternalOutput")

    # replicated-on-both-halves [128, 64] weights; W2 block-diag [128, 32]
    wname = ["Wenc", "Wself0", "Wself1", "Wsum0", "Wsum1", "W1", "idn"]
    w_d = {n: nc.dram_tensor(n, [128, 64], BF16, kind="ExternalInput") for n in wname}
    w_d["W2"] = nc.dram_tensor("W2", [128, 32], BF16, kind="ExternalInput")
    w_d["idnq"] = nc.dram_tensor("idnq", [128, 32], F32, kind="ExternalInput")
    bname = ["be", "b0", "b1", "bh"]
    b_d = {n: nc.dram_tensor(n, [128, 1], F32, kind="ExternalInput") for n in bname}

    FD = GRP
    Relu = mybir.ActivationFunctionType.Relu
    Copy = mybir.ActivationFunctionType.Copy
    ALU = mybir.AluOpType
    QUAD = [(0, 0, 0, 0), (0, 1, 0, 64), (1, 0, 64, 64), (1, 1, 64, 0)]
    # (in p-half, in f-half, rhs part base, out part base); out f-half = in f-half
    # after act: group at (ph, fh) lands at (out_base//64, fh) -> (0,1)/(1,0) swap

    with tile.TileContext(nc) as tc, contextlib.ExitStack() as ctx:
        wp = ctx.enter_context(tc.tile_pool(name="w", bufs=1))
        pool = ctx.enter_context(tc.tile_pool(name="p", bufs=3))
        psum = ctx.enter_context(tc.tile_pool(name="ps", bufs=1, space="PSUM"))

        W = {}
        for n in wname:
            W[n] = wp.tile([128, 64], BF16, tag=n, name=f"w_{n}")
            nc.sync.dma_start(W[n][:], w_d[n][:])
        W["W2"] = wp.tile([128, 32], BF16, tag="W2", name="w_W2")
        nc.sync.dma_start(W["W2"][:], w_d["W2"][:])
        W["idnq"] = wp.tile([128, 32], F32, tag="idnq", name="w_idnq")
        nc.sync.dma_start(W["idnq"][:], w_d["idnq"][:])
        BIAS = {}
        for n in bname:
            BIAS[n] = wp.tile([128, 1], F32, tag=n, name=f"b_{n}")
            nc.sync.dma_start(BIAS[n][:], b_d[n][:])

        def layer_mms(ps, wt, rhs_t):
            """4 concurrent K=64/M=64 matmuls (one per group) into ps[128,1024]."""
            for ph, fh, rb, ob in QUAD:
                nc.tensor.matmul(
                    ps[ob:ob + 64, fh * FD:(fh + 1) * FD],
                    wt[rb:rb + 64, :],
                    rhs_t[ph * 64:(ph + 1) * 64, fh * FD:(fh + 1) * FD],
                    start=True, stop=True, tile_position=(rb, ob),
                )

        for s in range(NSUP):
            obs_t = pool.tile([128, 2 * FD], BF16, tag="obs")
            nc.sync.dma_start(obs_t[:], obs_d[s])

            psE = psum.tile([128, 2 * FD], F32, tag="stg", bufs=3)
            layer_mms(psE, W["Wenc"], obs_t)
            h = pool.tile([128, 2 * FD], BF16, tag="h0")
            nc.vector.tensor_scalar(h[:], psE[:], BIAS["be"][:], 0.0,
                                    ALU.add, ALU.max)

            for r in range(NR):
                psS = psum.tile([128, NS_H], F32, tag="S")
                for hp, tp in ((0, 0), (64, 64)):
                    for sh in range(2):
                        rhs = h[hp:hp + 64, sh * FD:(sh + 1) * FD] \
                            .rearrange("p (S n) -> p n S", n=N)
                        outS = psS[hp:hp + 64, sh * NS_G:(sh + 1) * NS_G] \
                            .unsqueeze(1).broadcast_to([64, N, NS_G])
                        nc.tensor.matmul(outS, W["idn"][hp:hp + 64, :], rhs,
                                         start=True, stop=True,
                                         tile_position=(tp, tp))
                S2 = pool.tile([128, NS_H], BF16, tag="S2")
                nc.vector.tensor_copy(S2[:], psS[:])

                psR = psum.tile([128, 2 * FD], F32, tag="stg", bufs=3)
                for ph, fh, rb, ob in QUAD:
                    nc.tensor.matmul(
                        psR[ob:ob + 64, fh * FD:(fh + 1) * FD],
                        W[f"Wself{r}"][rb:rb + 64, :],
                        h[ph * 64:(ph + 1) * 64, fh * FD:(fh + 1) * FD],
                        start=True, stop=False, tile_position=(rb, ob),
                    )
                    sb = S2[ph * 64:(ph + 1) * 64, fh * NS_G:(fh + 1) * NS_G] \
                        .unsqueeze(2).broadcast_to([64, NS_G, N])
                    nc.tensor.matmul(
                        psR[ob:ob + 64, fh * FD:(fh + 1) * FD],
                        W[f"Wsum{r}"][rb:rb + 64, :], sb,
                        start=False, stop=True, tile_position=(rb, ob),
                    )
                h = pool.tile([128, 2 * FD], BF16, tag=f"h{1 + r}")
                nc.scalar.activation(h[:], psR[:], Relu, bias=BIAS[f"b{r}"][:])

            psH = psum.tile([128, 2 * FD], F32, tag="stg", bufs=3)
            layer_mms(psH, W["W1"], h)
            hid = pool.tile([128, 2 * FD], BF16, tag="hid")
            nc.vector.tensor_scalar(hid[:], psH[:], BIAS["bh"][:], 0.0,
                                    ALU.add, ALU.max)

            # out2: block-diag over partition pairs; two col positions.
            # q banks of even/odd super-iters pack into one [128, FD] bank
            # (partition halves) so the evacuation runs full-width half as often.
            k = s % 2
            qo = 64 * k
            if k == 0:
                pen_t = pool.tile([128, FD], F32, tag="pen")
                nc.sync.dma_start(pen_t[:], pen_d[s // 2])
                psQ = psum.tile([128, FD], F32, tag="q")
                pers = (pen_t, psQ)
            else:
                pen_t, psQ = pers
            nc.tensor.matmul(psQ[qo:qo + 32, :], W["W2"][:], hid[:, 0:FD],
                             start=True, stop=False, tile_position=(0, qo),
                             skip_group_check=True)
            nc.tensor.matmul(psQ[qo:qo + 32, :], W["idnq"][qo:qo + 32, :],
                             pen_t[qo:qo + 32, :],
                             start=False, stop=True, tile_position=(qo % 128 // 32 * 32, qo),
                             skip_group_check=True)
            nc.tensor.matmul(psQ[qo + 32:qo + 64, :], W["W2"][:], hid[:, FD:2 * FD],
                             start=True, stop=False, tile_position=(0, qo + 32),
                             skip_group_check=True)
            nc.tensor.matmul(psQ[qo + 32:qo + 64, :], W["idnq"][qo + 32:qo + 64, :],
                             pen_t[qo + 32:qo + 64, :],
                             start=False, stop=True,
                             tile_position=((qo + 32) % 128 // 32 * 32, qo + 32),
                             skip_group_check=True)
            if k == 1:
                q_sb = pool.tile([128, FD], BF16, tag="qsb")
                nc.scalar.activation(q_sb[:], psQ[:], Copy)
                nc.sync.dma_start(q_d[s // 2], q_sb[:])

    nc.compile()
    return nc


def _prep_host(obs, enc_w, enc_b, comm_w, comm_b, out_w1, out_b1, out_w2, out_b2,
               available_actions):
    """Build per-core input maps (packed layouts + derived weights)."""
    bf16 = ml_dtypes.bfloat16
    f32 = np.float32

    def rep(w):  # replicate [64, m] weight onto both partition halves
        return np.ascontiguousarray(np.concatenate([w, w], axis=0)
                                    .astype(f32)).astype(bf16)

    def bd(w):  # block-diag duplicate [k,m] -> [2k, 2m]
        k, m = w.shape
        o = np.zeros((2 * k, 2 * m), f32)
        o[:k, :m] = w
        o[k:, m:] = w
        return np.ascontiguousarray(o).astype(bf16)

    weights = {"Wenc": rep(enc_w), "W1": rep(out_w1), "W2": bd(out_w2),
               "idn": rep(np.eye(64, dtype=f32)),
               "idnq": np.ascontiguousarray(np.tile(np.eye(32, dtype=f32), (4, 1)))}
    for r in range(NR):
        wh = comm_w[r][:H].astype(f32)
        wm = comm_w[r][H:].astype(f32) / (N - 1)
        weights[f"Wself{r}"] = rep(wh - wm)
        weights[f"Wsum{r}"] = rep(wm)
    biases = {"be": enc_b, "b0": comm_b[0], "b1": comm_b[1], "bh": out_b1}
    biases = {k: np.concatenate([v, v]).astype(f32).reshape(128, 1)
              for k, v in biases.items()}

    rows = np.ascontiguousarray(obs.reshape(B * N, OBS))
    pen = np.where(available_actions.reshape(B * N, A) == 0,
                   f32(-1e10), out_b2.astype(f32)[None, :]).astype(f32)

    in_maps = []
    for c in range(NCORES):
        ro = rows[c * RPC:(c + 1) * RPC]
        # [NSUP, phalf, fhalf, row, feat] -> [NSUP, phalf*feat, fhalf*row]
        opk = ro.reshape(NSUP, 2, 2, GRP, OBS).transpose(0, 1, 4, 2, 3) \
                .reshape(NSUP, 128, SUP // 2).astype(bf16)
        pe = pen[c * RPC:(c + 1) * RPC]
        # q/pen partitions: [fhalf, phalf, action]
        ppk = pe.reshape(NSUP, 2, 2, GRP, A).transpose(0, 2, 1, 4, 3) \
                .reshape(NSUP // 2, 128, GRP).astype(f32)
        m = {"obs_pk": np.ascontiguousarray(opk),
             "pen_pk": np.ascontiguousarray(ppk)}
        m.update(weights)
        m.update(biases)
        in_maps.append(m)
    return in_maps


def _unpack_output(results):
    qs = []
    for r in results:
        qpk = np.asarray(r["q_pk"]).astype(np.float32)  # [NSUP//2, 128, GRP]
        q = qpk.reshape(NSUP, 2, 2, A, GRP).transpose(0, 2, 1, 4, 3) \
               .reshape(RPC, A)
        qs.append(q)
    return np.concatenate(qs, axis=0).reshape(B, N, A)


def run_on_device(in_maps, trace=False):
    from concourse.bass_utils import run_bass_kernel_spmd

    if "nc" not in _cache:
        _cache["nc"] = _build_device_program()
    return run_bass_kernel_spmd(_cache["nc"], in_maps,
                                core_ids=list(range(NCORES)), trace=trace)


def kernel(obs, enc_w, enc_b, comm_w, comm_b, out_w1, out_b1, out_w2, out_b2,
           available_actions):
    args = [np.asarray(x) for x in
            (obs, enc_w, enc_b, comm_w, comm_b, out_w1, out_b1, out_w2, out_b2,
             available_actions)]
    in_maps = _prep_host(*args)
    res = run_on_device(in_maps)
    return _unpack_output(res.results)



# revision 2
# speedup vs baseline: 1.1310x; 1.1310x over previous
"""CommNet forward kernel for 8 Trainium2 NeuronCores (v4).

Per-core structure (RPC = 65536 rows, supers of 2048 rows):
  * feature-major activations [128, 1024]: partition = ph*64 + feat,
    free = fh*512 + row; quad (ph, fh) at out-half ob = (ph^fh)*64, so
    fh=1 groups swap halves per layer (4 layers = identity).
  * 4-super waves, stage-interleaved: PE never waits on an evacuation
    round trip (3 stage slots >> evac+sem latency).
  * rounds: (self, sum) matmuls paired per quad (one open PSUM group per
    bank at a time); sum term uses stride-0 broadcast rhs of the DVE
    tensor_reduce'd agent sums.
  * out2 flipped (hid stationary, W2 moving, 16-free out): K=128
    matmuls against zero-padded W2 halves (W2_LO/W2_HI) so every matmul
    sits at tile_position (0,0) -- rapid tile_position alternation with
    tiny matmuls wedges the device.  Each matmul is its own closed PSUM
    group on disjoint columns; mask+out_b2 folded host-side.
  * evacs: enc/r0/r1 on Act (relu+bias), out1 + q on DVE.
"""

import contextlib
import sys

import numpy as np

sys.path.insert(0, "/opt/trn_rl_repo")

import ml_dtypes  # noqa: E402

B, N, OBS, H, A, NR = 16384, 32, 64, 64, 16, 2
NCORES = 8
RPC = B * N // NCORES   # rows per core = 65536

SUP = 2048              # rows per super-iteration
NSUP = RPC // SUP       # 32
FD = 512                # free columns per fh half
NS_F = FD // N          # samples per fh half = 16
W_WAVE = 4              # supers interleaved per wave

_cache = {}


def _build_device_program(nsup=NSUP):
    import concourse.bacc as bacc
    import concourse.mybir as mybir
    from concourse import tile

    F32 = mybir.dt.float32
    BF16 = mybir.dt.bfloat16

    nc = bacc.Bacc("TRN2", target_bir_lowering=False, debug=False)

    obs_d = nc.dram_tensor("obs_pk", [nsup, 128, 2 * FD], BF16, kind="ExternalInput")
    q_d = nc.dram_tensor("q_pk", [nsup // 2, 128, FD], BF16, kind="ExternalOutput")

    wname = ["Wenc", "Wself0", "Wself1", "Wsum0", "Wsum1", "W1"]
    w_d = {n: nc.dram_tensor(n, [128, 64], BF16, kind="ExternalInput") for n in wname}
    w_d["W2L"] = nc.dram_tensor("W2L", [128, 16], BF16, kind="ExternalInput")
    w_d["W2H"] = nc.dram_tensor("W2H", [128, 16], BF16, kind="ExternalInput")
    bname = ["be", "b0", "b1", "bh"]
    b_d = {n: nc.dram_tensor(n, [128, 1], F32, kind="ExternalInput") for n in bname}

    Relu = mybir.ActivationFunctionType.Relu
    X = mybir.AxisListType.X
    ALU = mybir.AluOpType
    QUAD = [(0, 0), (1, 0), (0, 1), (1, 1)]   # (ph, fh); ob = (ph^fh)*64

    with tile.TileContext(nc) as tc, contextlib.ExitStack() as ctx:
        wp = ctx.enter_context(tc.tile_pool(name="w", bufs=1))
        pool = ctx.enter_context(tc.tile_pool(name="p", bufs=3))
        psum = ctx.enter_context(tc.tile_pool(name="ps", bufs=1, space="PSUM"))

        W = {}
        for n in wname:
            W[n] = wp.tile([128, 64], BF16, tag=n, name=f"w_{n}")
            nc.sync.dma_start(W[n][:], w_d[n][:])
        for n in ("W2L", "W2H"):
            W[n] = wp.tile([128, 16], BF16, tag=n, name=f"w_{n}")
            nc.sync.dma_start(W[n][:], w_d[n][:])
        BIAS = {}
        for n in bname:
            BIAS[n] = wp.tile([128, 1], F32, tag=n, name=f"b_{n}")
            nc.sync.dma_start(BIAS[n][:], b_d[n][:])

        def layer_mms(ps_t, wt, rhs_t):
            """4 closed-group quadrant matmuls [64,512] into ps_t[128,1024]."""
            for ph, fh in QUAD:
                rb, ob = ph * 64, (ph ^ fh) * 64
                nc.tensor.matmul(
                    ps_t[ob:ob + 64, fh * FD:(fh + 1) * FD],
                    wt[rb:rb + 64, :],
                    rhs_t[ph * 64:(ph + 1) * 64, fh * FD:(fh + 1) * FD],
                    start=True, stop=True, tile_position=(rb, ob),
                )

        def round_mms(ps_t, r, h_t, s_t):
            """(self, sum) paired per quad: one open group per bank."""
            for ph, fh in QUAD:
                rb, ob = ph * 64, (ph ^ fh) * 64
                nc.tensor.matmul(
                    ps_t[ob:ob + 64, fh * FD:(fh + 1) * FD],
                    W[f"Wself{r}"][rb:rb + 64, :],
                    h_t[ph * 64:(ph + 1) * 64, fh * FD:(fh + 1) * FD],
                    start=True, stop=False, tile_position=(rb, ob),
                )
                sb = s_t[fh][ph * 64:(ph + 1) * 64, :] \
                    .unsqueeze(2).broadcast_to([64, NS_F, N])
                nc.tensor.matmul(
                    ps_t[ob:ob + 64, fh * FD:(fh + 1) * FD],
                    W[f"Wsum{r}"][rb:rb + 64, :], sb,
                    start=False, stop=True, tile_position=(rb, ob),
                )

        n_waves = nsup // W_WAVE
        for w in range(n_waves):
            ss = [w * W_WAVE + i for i in range(W_WAVE)]

            obs_t, h, S = {}, {}, {}
            for i, s in enumerate(ss):
                obs_t[i] = pool.tile([128, 2 * FD], BF16, tag="obs", bufs=8,
                                     name=f"obs_{s}")
                nc.sync.dma_start(obs_t[i][:], obs_d[s])

            def psum_tile(i, layer):
                return psum.tile([128, 2 * FD], F32, tag="p", bufs=3,
                                 name=f"ps_{layer}_{ss[i]}")

            def h_tile(i, layer):
                t = pool.tile([128, 2 * FD], BF16, tag="h", bufs=6,
                              name=f"h_{layer}_{ss[i]}")
                h[(i, layer)] = t
                return t

            def s_reduce(i, r):
                """DVE agent sums of h[(i, r)], one [128,16] tile per fh."""
                res = []
                with nc.allow_low_precision("agent-sum kept in bf16"):
                    for fh in (0, 1):
                        st = pool.tile([128, NS_F], BF16, tag="S", bufs=16,
                                       name=f"S_{r}_{fh}_{ss[i]}")
                        hin = h[(i, r)][:, fh * FD:(fh + 1) * FD] \
                            .rearrange("p (S n) -> p S n", n=N)
                        nc.vector.tensor_reduce(st[:], hin, X, ALU.add)
                        res.append(st)
                S[(i, r)] = res
                return res

            # ---- enc ------------------------------------------------------
            for i in range(W_WAVE):
                t = psum_tile(i, "e")
                layer_mms(t, W["Wenc"], obs_t[i])
                ht = h_tile(i, 0)
                nc.scalar.activation(ht[:], t[:], Relu, bias=BIAS["be"][:])
                s_reduce(i, 0)

            # ---- comm rounds ---------------------------------------------
            for r in range(NR):
                for i in range(W_WAVE):
                    t = psum_tile(i, f"r{r}")
                    round_mms(t, r, h[(i, r)], S[(i, r)])
                    ht = h_tile(i, r + 1)
                    nc.scalar.activation(ht[:], t[:], Relu,
                                         bias=BIAS[f"b{r}"][:])
                    if r + 1 < NR:
                        s_reduce(i, r + 1)

            # ---- out1 (evac on DVE) --------------------------------------
            for i in range(W_WAVE):
                t = psum_tile(i, "o1")
                layer_mms(t, W["W1"], h[(i, NR)])
                ht = h_tile(i, "hid")
                nc.vector.tensor_scalar(ht[:], t[:], BIAS["bh"][:], 0.0,
                                        ALU.add, ALU.max)

            # ---- out2: flipped, one PSUM group per psQ bank --------------
            for pr in (0, 1):
                psQ = psum.tile([128, FD], F32, tag="q", bufs=2,
                                name=f"psQ_{w}_{pr}")
                for k in (0, 1):
                    hid = h[(2 * pr + k, "hid")]
                    for fh in (0, 1):
                        for ph in (0, 1):
                            for b in range(4):
                                j = k * 16 + fh * 8 + ph * 4 + b
                                nc.tensor.matmul(
                                    psQ[:, 16 * j:16 * (j + 1)],
                                    hid[:, fh * FD + 128 * b:fh * FD + 128 * (b + 1)],
                                    W["W2L" if ph == 0 else "W2H"][:],
                                    start=True, stop=True,
                                    tile_position=(0, 0),
                                )
                q_sb = pool.tile([128, FD], BF16, tag="qsb", bufs=2,
                                 name=f"qsb_{w}_{pr}")
                nc.vector.tensor_copy(q_sb[:], psQ[:])
                nc.sync.dma_start(q_d[w * 2 + pr], q_sb[:])

    nc.compile()
    return nc


def _prep_host(obs, enc_w, enc_b, comm_w, comm_b, out_w1, out_b1, out_w2, out_b2,
               available_actions):
    """Build per-core input maps (packed layouts + derived weights)."""
    bf16 = ml_dtypes.bfloat16
    f32 = np.float32

    def rep(w):  # replicate [64, m] weight onto both partition halves
        return np.ascontiguousarray(np.concatenate([w, w], axis=0)
                                    .astype(f32)).astype(bf16)

    z16 = np.zeros((64, A), np.float32)
    w2f = out_w2.astype(f32)
    weights = {"Wenc": rep(enc_w), "W1": rep(out_w1),
               "W2L": np.ascontiguousarray(np.concatenate([w2f, z16])).astype(bf16),
               "W2H": np.ascontiguousarray(np.concatenate([z16, w2f])).astype(bf16)}
    for r in range(NR):
        wh = comm_w[r][:H].astype(f32)
        wm = comm_w[r][H:].astype(f32) / (N - 1)
        weights[f"Wself{r}"] = rep(wh - wm)
        weights[f"Wsum{r}"] = rep(wm)
    biases = {"be": enc_b, "b0": comm_b[0], "b1": comm_b[1], "bh": out_b1}
    biases = {k: np.concatenate([v, v]).astype(f32).reshape(128, 1)
              for k, v in biases.items()}

    rows = np.ascontiguousarray(obs.reshape(B * N, OBS))

    in_maps = []
    for c in range(NCORES):
        ro = rows[c * RPC:(c + 1) * RPC]
        opk = ro.reshape(NSUP, 2, 2, FD, OBS).transpose(0, 1, 4, 2, 3) \
                .reshape(NSUP, 128, 2 * FD).astype(bf16)
        m = {"obs_pk": np.ascontiguousarray(opk)}
        m.update(weights)
        m.update(biases)
        in_maps.append(m)
    return in_maps


def _unpack_output(results, out_b2, available_actions):
    qs = []
    for r in results:
        qpk = np.asarray(r["q_pk"]).astype(np.float32)  # [NSUP//2, 128, FD]
        # free col = k*256 + (fh*8 + ph*4 + b)*16 + a ; partition = row r128
        # global row = ((pr*2+k)*2048) + ph*1024 + fh*512 + b*128 + r128
        q = qpk.reshape(NSUP // 2, 128, 2, 2, 2, 4, A)  # [pr, r, k, fh, ph, b, a]
        q = q.transpose(0, 2, 4, 3, 5, 1, 6).reshape(RPC, A)
        qs.append(q)
    q = np.concatenate(qs, axis=0).reshape(B, N, A)
    q = q + out_b2.astype(np.float32)[None, None, :]
    return np.where(available_actions == 0, np.float32(-1e10), q)


def run_on_device(in_maps, trace=False):
    from concourse.bass_utils import run_bass_kernel_spmd

    if "nc" not in _cache:
        _cache["nc"] = _build_device_program()
    return run_bass_kernel_spmd(_cache["nc"], in_maps,
                                core_ids=list(range(NCORES)), trace=trace)


def kernel(obs, enc_w, enc_b, comm_w, comm_b, out_w1, out_b1, out_w2, out_b2,
           available_actions):
    args = [np.asarray(x) for x in
            (obs, enc_w, enc_b, comm_w, comm_b, out_w1, out_b1, out_w2, out_b2,
             available_actions)]
    in_maps = _prep_host(*args)
    res = run_on_device(in_maps)
    return _unpack_output(res.results, args[8], args[9])


# revision 3
# speedup vs baseline: 1.1885x; 1.0508x over previous
"""CommNet forward kernel for 8 Trainium2 NeuronCores (v5).

Per-core structure (RPC = 65536 rows, supers of 2048 rows):
  * feature-major activations [128, 1024]: partition = ph*64 + feat,
    free = fh*512 + row; fh=1 groups swap partition halves per layer.
  * every layer matmul is K=128 x M=128 at tile_position (0,0): weights
    are pre-expanded host-side into block-diagonal (fh=0) and
    anti-diagonal (fh=1) [128,128] variants, so one matmul per fh half
    replaces the two 64x64 quadrant matmuls (the cost of a matmul is set
    by its output free size, and rapidly alternating tile_position with
    small matmuls wedges the device).
  * comm round: h @ W_self + S @ W_sum; S (per-sample agent sum) via DVE
    tensor_reduce; the sum term broadcasts S with a stride-0 rhs AP into
    the same PSUM group as the self matmul (start/stop paired per bank).
  * out2 flipped: hid stationary, zero-padded W2L/W2H moving, K=128,
    16-free outputs, closed group per matmul; mask+out_b2 host-side.
  * evacs: enc/r0/out1 on Act (relu+bias), r1 split Act/DVE,
    S + q on DVE (gpsimd ops fail walrus codegen in this container).
  * 8-super waves, stage-interleaved for evac/S latency slack.
"""

import contextlib
import sys

import numpy as np

sys.path.insert(0, "/opt/trn_rl_repo")

import ml_dtypes  # noqa: E402

B, N, OBS, H, A, NR = 16384, 32, 64, 64, 16, 2
NCORES = 8
RPC = B * N // NCORES   # rows per core = 65536

SUP = 2048              # rows per super-iteration
NSUP = RPC // SUP       # 32
FD = 512                # free columns per fh half
NS_F = FD // N          # samples per fh half = 16
W_WAVE = 8              # supers interleaved per wave

_cache = {}

WNAMES = ["Wenc", "Wself0", "Wself1", "Wsum0", "Wsum1", "W1"]


def _derive_weights(enc_w, comm_w, out_w1, out_w2):
    bf16 = ml_dtypes.bfloat16
    f32 = np.float32

    def bd(w):  # block-diagonal [128,128]
        o = np.zeros((128, 128), f32)
        o[:64, :64] = w
        o[64:, 64:] = w
        return np.ascontiguousarray(o).astype(bf16)

    def ad(w):  # anti-diagonal [128,128]
        o = np.zeros((128, 128), f32)
        o[64:, :64] = w
        o[:64, 64:] = w
        return np.ascontiguousarray(o).astype(bf16)

    base = {"Wenc": enc_w.astype(f32), "W1": out_w1.astype(f32)}
    for r in range(NR):
        wh = comm_w[r][:H].astype(f32)
        wm = comm_w[r][H:].astype(f32) / (N - 1)
        base[f"Wself{r}"] = wh - wm
        base[f"Wsum{r}"] = wm
    weights = {}
    for n, w in base.items():
        weights[n + "_D"] = bd(w)
        weights[n + "_A"] = ad(w)
    z16 = np.zeros((64, A), f32)
    w2f = out_w2.astype(f32)
    weights["W2L"] = np.ascontiguousarray(np.concatenate([w2f, z16])).astype(bf16)
    weights["W2H"] = np.ascontiguousarray(np.concatenate([z16, w2f])).astype(bf16)
    return weights


def _build_device_program(nsup=NSUP, w_wave=W_WAVE):
    import concourse.bacc as bacc
    import concourse.mybir as mybir
    from concourse import tile

    F32 = mybir.dt.float32
    BF16 = mybir.dt.bfloat16

    nc = bacc.Bacc("TRN2", target_bir_lowering=False, debug=False)

    obs_d = nc.dram_tensor("obs_pk", [nsup, 128, 2 * FD], BF16, kind="ExternalInput")
    q_d = nc.dram_tensor("q_pk", [nsup // 2, 128, FD], BF16, kind="ExternalOutput")

    w_d = {}
    for n in WNAMES:
        for v in ("_D", "_A"):
            w_d[n + v] = nc.dram_tensor(n + v, [128, 128], BF16,
                                        kind="ExternalInput")
    for n in ("W2L", "W2H"):
        w_d[n] = nc.dram_tensor(n, [128, 16], BF16, kind="ExternalInput")
    bname = ["be", "b0", "b1", "bh"]
    b_d = {n: nc.dram_tensor(n, [128, 1], F32, kind="ExternalInput") for n in bname}

    Relu = mybir.ActivationFunctionType.Relu
    X = mybir.AxisListType.X
    ALU = mybir.AluOpType

    with tile.TileContext(nc) as tc, contextlib.ExitStack() as ctx:
        wp = ctx.enter_context(tc.tile_pool(name="w", bufs=1))
        pool = ctx.enter_context(tc.tile_pool(name="p", bufs=3))
        psum = ctx.enter_context(tc.tile_pool(name="ps", bufs=1, space="PSUM"))

        W = {}
        for n, d in w_d.items():
            W[n] = wp.tile(list(d.shape), BF16, tag=n, name=f"w_{n}")
            nc.sync.dma_start(W[n][:], d[:])
        BIAS = {}
        for n in bname:
            BIAS[n] = wp.tile([128, 1], F32, tag=n, name=f"b_{n}")
            nc.sync.dma_start(BIAS[n][:], b_d[n][:])

        def layer_mms(ps_t, wn, rhs_t):
            """One K=128 matmul per fh half into ps_t[128,1024]."""
            for fh, v in ((0, "_D"), (1, "_A")):
                nc.tensor.matmul(
                    ps_t[:, fh * FD:(fh + 1) * FD],
                    W[wn + v][:],
                    rhs_t[:, fh * FD:(fh + 1) * FD],
                    start=True, stop=True, tile_position=(0, 0),
                )

        def round_mms(ps_t, r, h_t, s_t):
            for fh, v in ((0, "_D"), (1, "_A")):
                nc.tensor.matmul(
                    ps_t[:, fh * FD:(fh + 1) * FD],
                    W[f"Wself{r}{v}"][:],
                    h_t[:, fh * FD:(fh + 1) * FD],
                    start=True, stop=False, tile_position=(0, 0),
                )
                sb = s_t[fh][:].unsqueeze(2).broadcast_to([128, NS_F, N])
                nc.tensor.matmul(
                    ps_t[:, fh * FD:(fh + 1) * FD],
                    W[f"Wsum{r}{v}"][:], sb,
                    start=False, stop=True, tile_position=(0, 0),
                )

        n_waves = nsup // w_wave
        for w in range(n_waves):
            ss = [w * w_wave + i for i in range(w_wave)]

            obs_t, h, S = {}, {}, {}
            for i, s in enumerate(ss):
                obs_t[i] = pool.tile([128, 2 * FD], BF16, tag="obs",
                                     bufs=2 * w_wave, name=f"obs_{s}")
                nc.sync.dma_start(obs_t[i][:], obs_d[s])

            def psum_tile(i, layer):
                return psum.tile([128, 2 * FD], F32, tag="p", bufs=3,
                                 name=f"ps_{layer}_{ss[i]}")

            def h_tile(i, layer):
                t = pool.tile([128, 2 * FD], BF16, tag="h", bufs=10,
                              name=f"h_{layer}_{ss[i]}")
                h[(i, layer)] = t
                return t

            def s_reduce(i, r):
                res = []
                with nc.allow_low_precision("agent-sum kept in bf16"):
                    for fh in (0, 1):
                        st = pool.tile([128, NS_F], BF16, tag="S", bufs=24,
                                       name=f"S_{r}_{fh}_{ss[i]}")
                        hin = h[(i, r)][:, fh * FD:(fh + 1) * FD] \
                            .rearrange("p (S n) -> p S n", n=N)
                        nc.vector.tensor_reduce(st[:], hin, X, ALU.add)
                        res.append(st)
                S[(i, r)] = res
                return res

            # ---- enc ------------------------------------------------------
            for i in range(w_wave):
                t = psum_tile(i, "e")
                layer_mms(t, "Wenc", obs_t[i])
                ht = h_tile(i, 0)
                nc.scalar.activation(ht[:], t[:], Relu, bias=BIAS["be"][:])
                s_reduce(i, 0)

            # ---- comm rounds (r0 evac Act, r1 evac GpSimd) ---------------
            for r in range(NR):
                for i in range(w_wave):
                    t = psum_tile(i, f"r{r}")
                    round_mms(t, r, h[(i, r)], S[(i, r)])
                    ht = h_tile(i, r + 1)
                    if r == 0:
                        nc.scalar.activation(ht[:], t[:], Relu,
                                             bias=BIAS[f"b{r}"][:])
                    else:
                        nc.scalar.activation(ht[:, 0:FD], t[:, 0:FD], Relu,
                                             bias=BIAS[f"b{r}"][:])
                        nc.vector.tensor_scalar(ht[:, FD:2 * FD],
                                                t[:, FD:2 * FD],
                                                BIAS[f"b{r}"][:], 0.0,
                                                ALU.add, ALU.max)
                    if r + 1 < NR:
                        s_reduce(i, r + 1)

            # ---- out1 (evac Act) -----------------------------------------
            for i in range(w_wave):
                t = psum_tile(i, "o1")
                layer_mms(t, "W1", h[(i, NR)])
                ht = h_tile(i, "hid")
                nc.scalar.activation(ht[:], t[:], Relu, bias=BIAS["bh"][:])

            # ---- out2 ----------------------------------------------------
            for pr in range(w_wave // 2):
                psQ = psum.tile([128, FD], F32, tag="q", bufs=2,
                                name=f"psQ_{w}_{pr}")
                for k in (0, 1):
                    hid = h[(2 * pr + k, "hid")]
                    for fh in (0, 1):
                        for ph in (0, 1):
                            for b in range(4):
                                j = k * 16 + fh * 8 + ph * 4 + b
                                nc.tensor.matmul(
                                    psQ[:, 16 * j:16 * (j + 1)],
                                    hid[:, fh * FD + 128 * b:fh * FD + 128 * (b + 1)],
                                    W["W2L" if ph == 0 else "W2H"][:],
                                    start=True, stop=True,
                                    tile_position=(0, 0),
                                )
                q_sb = pool.tile([128, FD], BF16, tag="qsb", bufs=2,
                                 name=f"qsb_{w}_{pr}")
                nc.vector.tensor_copy(q_sb[:], psQ[:])
                nc.sync.dma_start(q_d[w * (w_wave // 2) + pr], q_sb[:])

    nc.compile()
    return nc


def _prep_host(obs, enc_w, enc_b, comm_w, comm_b, out_w1, out_b1, out_w2, out_b2,
               available_actions):
    bf16 = ml_dtypes.bfloat16
    f32 = np.float32

    weights = _derive_weights(enc_w, comm_w, out_w1, out_w2)
    biases = {"be": enc_b, "b0": comm_b[0], "b1": comm_b[1], "bh": out_b1}
    biases = {k: np.concatenate([v, v]).astype(f32).reshape(128, 1)
              for k, v in biases.items()}

    rows = np.ascontiguousarray(obs.reshape(B * N, OBS))

    in_maps = []
    for c in range(NCORES):
        ro = rows[c * RPC:(c + 1) * RPC]
        opk = ro.reshape(NSUP, 2, 2, FD, OBS).transpose(0, 1, 4, 2, 3) \
                .reshape(NSUP, 128, 2 * FD).astype(bf16)
        m = {"obs_pk": np.ascontiguousarray(opk)}
        m.update(weights)
        m.update(biases)
        in_maps.append(m)
    return in_maps


def _unpack_output(results, out_b2, available_actions):
    qs = []
    for r in results:
        qpk = np.asarray(r["q_pk"]).astype(np.float32)  # [NSUP//2, 128, FD]
        q = qpk.reshape(NSUP // 2, 128, 2, 2, 2, 4, A)  # [pr, r, k, fh, ph, b, a]
        q = q.transpose(0, 2, 4, 3, 5, 1, 6).reshape(RPC, A)
        qs.append(q)
    q = np.concatenate(qs, axis=0).reshape(B, N, A)
    q = q + out_b2.astype(np.float32)[None, None, :]
    return np.where(available_actions == 0, np.float32(-1e10), q)


def run_on_device(in_maps, trace=False):
    from concourse.bass_utils import run_bass_kernel_spmd

    if "nc" not in _cache:
        _cache["nc"] = _build_device_program()
    return run_bass_kernel_spmd(_cache["nc"], in_maps,
                                core_ids=list(range(NCORES)), trace=trace)


def kernel(obs, enc_w, enc_b, comm_w, comm_b, out_w1, out_b1, out_w2, out_b2,
           available_actions):
    args = [np.asarray(x) for x in
            (obs, enc_w, enc_b, comm_w, comm_b, out_w1, out_b1, out_w2, out_b2,
             available_actions)]
    in_maps = _prep_host(*args)
    res = run_on_device(in_maps)
    return _unpack_output(res.results, args[8], args[9])
